# revision 21
# baseline (speedup 1.0000x reference)
"""Bass/Tile kernel for nn_FMMPreWork on TRN2, one sample per NeuronCore.

Layouts
-------
A2: channel-major tensors on 112 partitions:
  p in [0,48):   channel c=p,    spatial half 0 (h in [0,64))
  p in [64,112): channel c=p-64, spatial half 1 (h in [64,128))
  partitions [48,64) are a gap (enables 2-way PE row+col packing).
Padded A2 tile [112, 66, 130]: interior [:, 1:65, 1:129]; local row r is
  h = r-1 (half0) / h = 63+r (half1); col j is w = j-1.
Compact A2 tile [112, 64, 128].
FFT runs per 4-channel group in [spatial, (ch, spatial)] layouts staged
through DRAM bounce buffers ((h, c, w)-ordered).

The frequency-domain channel mix (M yf + b) is moved to the spatial domain:
  M yf + b = FFT2(U low + V lowrev + b delta_00),  U/V complex [48,48].
Attention uses gram-folded normalization: q/k L2 norms come from the same
PSUM-accumulated gram products, so q/k are never explicitly normalized.
"""
import math
from itertools import product

import ml_dtypes
import numpy as np

import concourse.bass as bass
import concourse.mybir as mybir
import concourse.tile as tile
from concourse import bacc

DT = mybir.dt
BF = DT.bfloat16
FR = DT.float32r
F32 = DT.float32
AF = mybir.ActivationFunctionType
OP = mybir.AluOpType
AX = mybir.AxisListType

C = 48
H = W = 128
HH = 64
HW = H * W
NP = 112
NF = 65
GROUP = 8
HEADS = 4
BN_EPS = 1e-5
MASKVAL = -60.0

bf16 = ml_dtypes.bfloat16

_WSHAPES = {
    'fd_w': (72, 48), 'bn_g': (72,), 'bn_b': (72,), 'bn_m': (72,),
    'bn_v': (72,), 'fh_w1': (48, 1, 1, 3), 'fh_b1': (48,),
    'fh_w2': (48, 1, 3, 1), 'fh_b2': (48,), 'fl_w': (96, 96), 'fl_b': (96,),
    'hfa_qw': (48, 48), 'hfa_kvw': (96, 48), 'hfa_qdw': (48, 1, 3, 3),
    'hfa_kvdw': (96, 1, 3, 3), 'hfa_pw': (48, 48), 'hfa_t': (4, 1, 1),
    'lfa_qw': (48, 48), 'lfa_kvw': (96, 48), 'lfa_qdw': (48, 1, 3, 3),
    'lfa_kvdw': (96, 1, 3, 3), 'lfa_pw': (48, 48), 'lfa_t': (4, 1, 1),
    'fp_w': (48, 96), 'fp_b': (48,),
}


def a2vec(v48, dtype=np.float32):
    out = np.zeros((NP, 1), dtype=dtype)
    out[0:48, 0] = v48
    out[64:112, 0] = v48
    return out


def a2mat(m, dtype):
    x = np.asarray(m)
    out = np.zeros((NP,) + x.shape[1:], np.float64)
    out[0:48] = x
    out[64:112] = x
    return out.astype(dtype)


def a2mat64(m, dtype):
    """[48, 48] lhsT -> [112, 64] with zero-padded out-channels 48:64 so
    matmuls at tile (0,0) cover psum partitions 0..64 (gap stays zeroed)."""
    x = np.asarray(m)
    out = np.zeros((NP, 64), np.float64)
    out[0:48, 0:48] = x
    out[64:112, 0:48] = x
    return out.astype(dtype)


def host_consts(w):
    """Derived constant inputs. w: dict of np.float32 weight arrays."""
    cst = {}
    f8 = np.float64

    s = 1.0 / np.sqrt(w['bn_v'].astype(f8) + BN_EPS)
    fdw_f = (w['bn_g'].astype(f8) * s)[:, None] * w['fd_w'].astype(f8)
    fdb_f = w['bn_b'].astype(f8) - w['bn_m'].astype(f8) * w['bn_g'].astype(f8) * s
    cst['fdw_fT'] = np.ascontiguousarray(fdw_f.T).astype(np.float32)
    cst['fdb_f'] = fdb_f.astype(np.float32).reshape(72, 1)
    ps = np.zeros((NP, 48), np.float32)
    for c in range(48):
        ps[c, c] = 1.0 / HW
        ps[64 + c, c] = 1.0 / HW
    cst['pairsum'] = ps
    cst['I72f'] = np.eye(72, dtype=np.float32)
    cst['I48f'] = np.eye(48, dtype=np.float32)
    cst['I128b'] = np.eye(128, dtype=np.float32).astype(bf16)

    cst['fh_w1v'] = np.concatenate(
        [a2vec(w['fh_w1'][:, 0, 0, j]) for j in range(3)], axis=1)
    cst['fh_w2v'] = np.concatenate(
        [a2vec(w['fh_w2'][:, 0, i, 0]) for i in range(3)], axis=1)
    cst['fh_b1v'] = a2vec(w['fh_b1'])
    cst['fh_b2v'] = a2vec(w['fh_b2'])

    def fuse(w1, dw, rows):
        t = np.zeros((NP, 9, 64), np.float64)
        for i in range(3):
            for j in range(3):
                Wd = dw[rows, 0, i, j].astype(f8)[:, None] * w1[rows, :].astype(f8)
                t[0:48, i * 3 + j, 0:48] = Wd.T
                t[64:112, i * 3 + j, 0:48] = Wd.T
        return t.astype(bf16)
    for pre, qw, kvw, qdw, kvdw in (
            ('h', w['hfa_qw'], w['hfa_kvw'], w['hfa_qdw'], w['hfa_kvdw']),
            ('l', w['lfa_qw'], w['lfa_kvw'], w['lfa_qdw'], w['lfa_kvdw'])):
        cst[pre + 'qT'] = fuse(qw, qdw, slice(0, 48))
        cst[pre + 'kT'] = fuse(kvw, kvdw, slice(0, 48))
        cst[pre + 'vT'] = fuse(kvw, kvdw, slice(48, 96))

    A = w['fl_w'][0:C, 0:C].astype(f8)
    B = w['fl_w'][0:C, C:2 * C].astype(f8)
    Cb = w['fl_w'][C:2 * C, 0:C].astype(f8)
    D = w['fl_w'][C:2 * C, C:2 * C].astype(f8)
    for name, M in (('UrT', (A + D) / 2), ('UiT', (Cb - B) / 2),
                    ('VrT', (A - D) / 2), ('ViT', (Cb + B) / 2)):
        cst[name] = a2mat64(M.T, bf16)
    cst['flbP'] = w['fl_b'][0:C].astype(np.float32).reshape(48, 1)
    cst['flbQ'] = w['fl_b'][C:2 * C].astype(np.float32).reshape(48, 1)

    n = np.arange(128)
    th = 2 * np.pi * np.outer(n, n) / 128.0
    thw = 2 * np.pi * np.outer(n, np.arange(NF)) / 128.0
    CH = np.cos(th); SH = np.sin(th)
    CW = np.cos(thw); SW = np.sin(thw)
    cst['CH'] = CH.astype(bf16); cst['SH'] = SH.astype(bf16)
    cst['SHn'] = (-SH).astype(bf16)
    cst['CW'] = CW.astype(bf16); cst['SW'] = SW.astype(bf16)
    cst['SWn'] = (-SW).astype(bf16)
    ck = np.ones(NF); ck[1:NF - 1] = 2.0
    IWc = (ck[None, :] / 128.0) * np.cos(thw)
    IWs = -(ck[None, :] / 128.0) * np.sin(thw)
    cst['IWcT'] = np.ascontiguousarray(IWc.T).astype(bf16)
    cst['IWsT'] = np.ascontiguousarray(IWs.T).astype(bf16)
    cst['IWcTn'] = np.ascontiguousarray(-IWc.T).astype(bf16)
    cst['cH2'] = (CH / 128.0).astype(bf16)
    cst['sH2'] = (SH / 128.0).astype(bf16)

    cst['Ia2'] = a2mat64(np.eye(48), bf16)
    d1 = np.zeros((NP, 3, 64), np.float64)
    d2 = np.zeros((NP, 3, 64), np.float64)
    for jj in range(3):
        d1[0:48, jj, 0:48] = np.diag(w['fh_w1'][:, 0, 0, jj].astype(f8))
        d1[64:112, jj, 0:48] = d1[0:48, jj, 0:48]
        d2[0:48, jj, 0:48] = np.diag(w['fh_w2'][:, 0, jj, 0].astype(f8))
        d2[64:112, jj, 0:48] = d2[0:48, jj, 0:48]
    cst['fhD1'] = d1.astype(bf16)
    cst['fhD2'] = d2.astype(bf16)
    cst['ones11'] = np.ones((1, 1), np.float32)
    t9 = np.zeros((72, 9), np.float32)
    for j in range(72):
        t9[j, j % 9] = 1.0
    cst['T9'] = t9
    selc = np.zeros((72, NP), np.float32)
    for c in range(48):
        for j in range(72):
            if j // 9 == c // 6:
                selc[j, c] = 1.0
                selc[j, 64 + c] = 1.0
    cst['SelC'] = selc

    cst['Ia2f'] = a2mat(np.eye(48), np.float32)
    hm = np.arange(48) // (C // HEADS)
    cst['mask48'] = np.where(hm[:, None] == hm[None, :], 0.0,
                             MASKVAL).astype(np.float32)
    cst['t_h'] = w['hfa_t'][hm, 0, 0].astype(np.float32).reshape(48, 1)
    cst['t_l'] = w['lfa_t'][hm, 0, 0].astype(np.float32).reshape(48, 1)
    cst['WpT_h'] = np.ascontiguousarray(w['hfa_pw'].T).astype(np.float32)
    cst['WpT_l'] = np.ascontiguousarray(w['lfa_pw'].T).astype(np.float32)

    cst['fpw_hT'] = a2mat64(w['fp_w'][:, 0:C].astype(f8).T, bf16)
    cst['fpw_lT'] = a2mat64(w['fp_w'][:, C:2 * C].astype(f8).T, bf16)
    cst['fp_bv'] = a2vec(w['fp_b'])
    return cst


def _ap(base, offset_delta, dims):
    return bass.AP(tensor=base.tensor, offset=base.offset + offset_delta,
                   ap=dims)


def build_module(taps=()):
    nc = bacc.Bacc('TRN2', target_bir_lowering=False, debug=False,
                   num_devices=8)
    x_d = nc.dram_tensor("x", [C, H, W], F32, kind="ExternalInput")
    out_d = nc.dram_tensor("out", [C, H, W], F32, kind="ExternalOutput")

    shapes = host_consts({k: np.zeros(s, np.float32)
                          for k, s in _WSHAPES.items()})
    cst_d = {}
    for k, v in shapes.items():
        dt = BF if v.dtype == bf16 else F32
        cst_d[k] = nc.dram_tensor(k, list(v.shape), dt, kind="ExternalInput")

    tap_d = {}
    for t in taps:
        shape = {'w72': [1, 72]}.get(t, [C, H, W])
        tap_d[t] = nc.dram_tensor("tap_" + t, shape, F32,
                                  kind="ExternalOutput")

    with tile.TileContext(nc) as tc:
        _build(nc, tc, x_d, out_d, cst_d, tap_d)
    nc.compile()
    return nc


def _build(nc, tc, x_d, out_d, cst_d, tap_d):
    import contextlib
    with contextlib.ExitStack() as ctx:
        ep = ctx.enter_context
        cp = ep(tc.tile_pool(name="consts", bufs=1))
        dram = ep(tc.tile_pool(name="dram", bufs=1, space="DRAM"))
        pads = ep(tc.tile_pool(name="pads", bufs=1))
        a2c = ep(tc.tile_pool(name="a2c", bufs=1))
        scr = ep(tc.tile_pool(name="scr", bufs=2))
        scrF = ep(tc.tile_pool(name="scrF", bufs=4))
        scrFi = ep(tc.tile_pool(name="scrFi", bufs=3))
        sm = ep(tc.tile_pool(name="sm", bufs=1))
        ps_mm = ep(tc.tile_pool(name="psmm", bufs=3, space="PSUM"))
        ps_tr = ep(tc.tile_pool(name="pstr", bufs=3, space="PSUM"))
        ps_g = ep(tc.tile_pool(name="psg", bufs=1, space="PSUM"))
        _body(nc, x_d, out_d, cst_d, tap_d,
              cp, dram, pads, a2c, scr, scrF, scrFi, sm, ps_mm, ps_tr, ps_g)


def _body(nc, x_d, out_d, cst_d, tap_d,
          cp, dram, pads, a2c, scr, scrF, scrFi, sm, ps_mm, ps_tr, ps_g):
    g = nc.gpsimd

    # ============ P0: raw x in SBUF (A2 halves with 1-row overlap) ==========
    # xraw[0:48, r, :]  = x[c, r, :]      for r in [0, 65)   (h = 0..64)
    # xraw[64:112, r, :] = x[c, 63+r, :]  for r in [0, 65)   (h = 63..127)
    # Time-shares the hfpad buffer (tag p2): every xraw read happens long
    # before hfpad is written.  Flat per-partition layout keeps the input
    # DMA a single contiguous 16.6 KB run per partition.
    XPITCH = 66 * 130
    xrawT = pads.tile([NP, 66, 130], BF, tag="p2")

    def xrv(ps, rs, cs=slice(0, W)):
        return _ap(xrawT[...],
                   ps.start * XPITCH + rs.start * W + cs.start,
                   [[XPITCH, ps.stop - ps.start], [W, rs.stop - rs.start],
                    [1, cs.stop - cs.start]])

    g.dma_start(out=xrv(slice(0, 48), slice(0, 65)), in_=x_d[:, 0:65, :])
    g.dma_start(out=xrv(slice(64, 112), slice(0, 65)), in_=x_d[:, 63:128, :])

    cc = {}
    order = ['hkT', 'hvT', 'pairsum', 'fdw_fT', 'fdb_f', 'I72f', 'ones11',
             'T9', 'SelC', 'Ia2', 'fh_b1v', 'fh_b2v', 'fhD1', 'fhD2',
             'I128b', 'I48f']
    keys = order + [k for k in cst_d if k not in order]
    qs = [nc.sync, nc.scalar, nc.gpsimd]
    for i, k in enumerate(keys):
        d = cst_d[k]
        t = cp.tile(list(d.shape), d.dtype, tag="c_" + k)
        qs[i % 3].dma_start(out=t[...], in_=d[...])
        cc[k] = t

    def tap_a2(name, src):
        """compact A2 [112, 64, 128] -> tap [C, H, W] f32"""
        if name not in tap_d:
            return
        d = tap_d[name]
        for hb, p0 in ((0, 0), (1, 64)):
            g.dma_start(out=d[:, hb * HH:(hb + 1) * HH, :],
                        in_=src[p0:p0 + 48, :, :])

    def tap_pad(name, src):
        if name not in tap_d:
            return
        d = tap_d[name]
        for hb, p0 in ((0, 0), (1, 64)):
            g.dma_start(out=d[:, hb * HH:(hb + 1) * HH, :],
                        in_=src[p0:p0 + 48, 1:65, 1:129])

    def evac(dst_ap, src_ap, which):
        if which == 0:
            nc.vector.tensor_copy(dst_ap, src_ap)
        else:
            nc.scalar.copy(dst_ap, src_ap)

    def tap_conv(srcpad, tapsT, taps, evac_fn):
        """taps: list of (t_idx, dh, dw); evac_fn(ch, psum)."""
        nt = len(taps)
        for ch in range(16):
            pt = ps_mm.tile([NP, 512], F32, tag="mm")
            for n, (t, i, j) in enumerate(taps):
                for p0, mo in ((0, 64), (64, 48)):
                    rhs = srcpad[p0:p0 + 48, 4 * ch + i:4 * ch + i + 4,
                                 j:j + 128]
                    nc.tensor.matmul(pt[p0:p0 + mo, :],
                                     tapsT[p0:p0 + 48, t, 0:mo], rhs,
                                     start=(n == 0), stop=(n == nt - 1),
                                     tile_position=(p0, p0),
                                     skip_group_check=True)
            evac_fn(ch, pt)

    TAPS9 = [(t, i, j) for t, (i, j) in
             enumerate(product(range(3), range(3)))]

    def conv_taps(dst, srcpad, tapsT):
        tap_conv(srcpad, tapsT, TAPS9,
                 lambda ch, pt: evac(dst[:, 4 * ch:4 * ch + 4, :], pt[...],
                                     ch % 2))


    def pad_border(t):
        """Zero borders only. Gap partitions [48:64] are left unwritten:
        conv rhs reads use [0:48]/[64:112] slices and never touch them."""
        nc.vector.memset(t[0:48, 0:1, :], 0.0)
        nc.vector.memset(t[64:112, 65:66, :], 0.0)
        g.memset(t[:, :, 0:1], 0.0)
        g.memset(t[:, :, 129:130], 0.0)

    # ============ P1: padded x (built on-chip from xraw) ============
    xpadR = pads.tile([NP, 66, 130], BF, tag="p0")
    xpad0 = pads.tile([NP, 66, 130], BF, tag="px")
    h0, h1 = slice(0, 48), slice(64, 112)
    # reflect pad, half0 (local rows r=0..65 are h=-1..64)
    nc.vector.tensor_copy(xpadR[0:48, 1:66, 1:129], xrv(h0, slice(0, 65)))
    nc.scalar.copy(xpadR[64:112, 0:65, 1:129], xrv(h1, slice(0, 65)))
    nc.vector.tensor_copy(xpadR[0:48, 0:1, 1:129], xrv(h0, slice(1, 2)))
    nc.vector.tensor_copy(xpadR[64:112, 65:66, 1:129],
                          xrv(h1, slice(63, 64)))
    g.tensor_copy(xpadR[0:48, 1:66, 0:1],
                  xrv(h0, slice(0, 65), slice(1, 2)))
    g.tensor_copy(xpadR[0:48, 1:66, 129:130],
                  xrv(h0, slice(0, 65), slice(126, 127)))
    g.tensor_copy(xpadR[64:112, 0:65, 0:1],
                  xrv(h1, slice(0, 65), slice(1, 2)))
    g.tensor_copy(xpadR[64:112, 0:65, 129:130],
                  xrv(h1, slice(0, 65), slice(126, 127)))
    g.tensor_copy(xpadR[0:48, 0:1, 0:1], xrv(h0, slice(1, 2), slice(1, 2)))
    g.tensor_copy(xpadR[0:48, 0:1, 129:130],
                  xrv(h0, slice(1, 2), slice(126, 127)))
    g.tensor_copy(xpadR[64:112, 65:66, 0:1],
                  xrv(h1, slice(63, 64), slice(1, 2)))
    g.tensor_copy(xpadR[64:112, 65:66, 129:130],
                  xrv(h1, slice(63, 64), slice(126, 127)))
    # zero pad
    pad_border(xpad0)
    nc.vector.tensor_copy(xpad0[0:48, 1:66, 1:129], xrv(h0, slice(0, 65)))
    nc.scalar.copy(xpad0[64:112, 0:65, 1:129], xrv(h1, slice(0, 65)))

    # ============ P1.5: hfa k/v convs (only need xpad0; fill PE early) ====
    kh = a2c.tile([NP, HH, W], BF, tag="s1")
    vh = a2c.tile([NP, HH, W], BF, tag="s5")
    conv_taps(kh, xpad0, cc['hkT'])
    conv_taps(vh, xpad0, cc['hvT'])

    # ============ P2: mean -> softmax -> wvec ============
    part = sm.tile([NP, 1], F32, tag="part")
    nc.vector.memset(part[32:64, :], 0.0)
    nc.vector.tensor_reduce(part[0:48, :], xrv(h0, slice(0, 64)), AX.XY,
                            OP.add)
    nc.vector.tensor_reduce(part[64:112, :], xrv(h1, slice(1, 65)), AX.XY,
                            OP.add)
    pmean = ps_g.tile([48, 72], F32, tag="gsm")
    nc.tensor.matmul(pmean[:, 0:1], cc['pairsum'][...], part[...],
                     start=True, stop=True)
    mean48 = sm.tile([48, 1], F32, tag="mean48")
    nc.vector.tensor_copy(mean48[...], pmean[:, 0:1])
    pfd = ps_g.tile([72, 72], F32, tag="gsm")
    nc.tensor.matmul(pfd[:, 0:1], cc['fdw_fT'][...], mean48[...],
                     start=True, stop=True)
    smin = sm.tile([72, 1], F32, tag="smin")
    nc.vector.tensor_scalar(smin[...], pfd[:, 0:1], cc['fdb_f'][...], None,
                            OP.add)
    ptr72 = ps_g.tile([1, 72], F32, tag="gsm")
    nc.tensor.matmul(ptr72[...], smin[...], cc['I72f'][...],
                     start=True, stop=True)
    s72 = sm.tile([1, 72], F32, tag="s72")
    nc.vector.tensor_copy(s72[...], ptr72[...])
    s72v = s72[...].rearrange("p (g k) -> p g k", g=GROUP)
    mx8 = sm.tile([1, 8], F32, tag="mx8")
    nc.vector.reduce_max(mx8[...], s72v, AX.X)
    e72 = sm.tile([1, 72], F32, tag="e72")
    mxb = _ap(mx8[...], 0, [list(mx8[...].ap[0]), [1, 8], [0, 9]])
    nc.vector.tensor_sub(e72[...].rearrange("p (g k) -> p g k", g=GROUP),
                         s72v, mxb)
    nc.scalar.activation(e72[...], e72[...], AF.Exp)
    sum8 = sm.tile([1, 8], F32, tag="sum8")
    nc.vector.reduce_sum(sum8[...],
                         e72[...].rearrange("p (g k) -> p g k", g=GROUP),
                         AX.X)
    nc.vector.reciprocal(sum8[...], sum8[...])
    w72 = sm.tile([1, 72], F32, tag="w72")
    sumb = _ap(sum8[...], 0, [list(sum8[...].ap[0]), [1, 8], [0, 9]])
    nc.vector.tensor_mul(w72[...].rearrange("p (g k) -> p g k", g=GROUP),
                         e72[...].rearrange("p (g k) -> p g k", g=GROUP),
                         sumb)
    if 'w72' in tap_d:
        nc.sync.dma_start(out=tap_d['w72'][...], in_=w72[...])
    pw72c = ps_g.tile([72, 9], F32, tag="gsm")
    nc.tensor.matmul(pw72c[:, 0:1], w72[...], cc['ones11'][...],
                     start=True, stop=True)
    w72col = sm.tile([72, 1], F32, tag="w72col")
    nc.vector.tensor_copy(w72col[...], pw72c[:, 0:1])
    rhs9 = sm.tile([72, 9], F32, tag="rhs9")
    nc.vector.tensor_mul(rhs9[...], cc['T9'][...],
                         _ap(w72col[...], 0, [list(w72col[...].ap[0]),
                                              [0, 9]]))
    pwv = ps_g.tile([NP, 9], F32, tag="gsm")
    nc.tensor.matmul(pwv[...], cc['SelC'][...], rhs9[...],
                     start=True, stop=True)
    wvec = sm.tile([NP, 9], F32, tag="wvec")
    nc.vector.tensor_copy(wvec[...], pwv[...])

    # ============ P3: involution (depthwise -> DVE) + high ============
    # low[c, r, w] = sum_t wvec[c, t] * xpadR[c, r+i, w+j], t=(i, j).
    # Per-partition scalar multiply-accumulate; frees the PE entirely.
    low = a2c.tile([NP, HH, W], BF, tag="s2")
    nc.vector.tensor_scalar(low[...], xpadR[:, 0:64, 0:128], wvec[:, 0:1],
                            None, OP.mult)
    for t, (i, j) in enumerate(product(range(3), range(3))):
        if t == 0:
            continue
        nc.vector.scalar_tensor_tensor(low[...],
                                       xpadR[:, i:i + 64, j:j + 128],
                                       wvec[:, t:t + 1], low[...],
                                       OP.mult, OP.add)
    tap_a2('low', low)

    highpad = pads.tile([NP, 66, 130], BF, tag="p1")
    pad_border(highpad)
    nc.vector.scalar_tensor_tensor(
        highpad[0:48, 1:65, 1:129], low[0:48, :, :], -1.0,
        xrv(h0, slice(0, 64)), OP.mult, OP.add)
    nc.vector.scalar_tensor_tensor(
        highpad[64:112, 1:65, 1:129], low[64:112, :, :], -1.0,
        xrv(h1, slice(1, 65)), OP.mult, OP.add)
    nc.sync.dma_start(out=highpad[0:48, 65:66, 1:129],
                in_=highpad[64:112, 1:2, 1:129])
    nc.sync.dma_start(out=highpad[64:112, 0:1, 1:129],
                in_=highpad[0:48, 64:65, 1:129])
    tap_pad('high', highpad)

    # ============ P4: fmgm_high (depthwise -> DVE) ============
    y1pad = pads.tile([NP, 66, 130], BF, tag="p0")
    pad_border(y1pad)
    y1i = y1pad[:, 1:65, 1:129]
    nc.vector.tensor_scalar(y1i, highpad[:, 1:65, 0:128], cc['fh_w1v'][:, 0:1],
                            cc['fh_b1v'][...], OP.mult, OP.add)
    for jj in (1, 2):
        nc.vector.scalar_tensor_tensor(y1i, highpad[:, 1:65, jj:jj + 128],
                                       cc['fh_w1v'][:, jj:jj + 1], y1i,
                                       OP.mult, OP.add)
    nc.sync.dma_start(out=y1pad[0:48, 65:66, 1:129], in_=y1pad[64:112, 1:2, 1:129])
    nc.sync.dma_start(out=y1pad[64:112, 0:1, 1:129], in_=y1pad[0:48, 64:65, 1:129])

    hfpad = pads.tile([NP, 66, 130], BF, tag="p2")
    pad_border(hfpad)
    hint = hfpad[:, 1:65, 1:129]
    nc.vector.tensor_scalar(hint, y1pad[:, 0:64, 1:129], cc['fh_w2v'][:, 0:1],
                            None, OP.mult)
    for ii in (1, 2):
        nc.vector.scalar_tensor_tensor(hint, y1pad[:, ii:ii + 64, 1:129],
                                       cc['fh_w2v'][:, ii:ii + 1], hint,
                                       OP.mult, OP.add)
    nc.scalar.activation(hint, hint, AF.Gelu, bias=cc['fh_b2v'][...])
    nc.vector.tensor_mul(hint, hint, highpad[:, 1:65, 1:129])
    nc.sync.dma_start(out=hfpad[0:48, 65:66, 1:129], in_=hfpad[64:112, 1:2, 1:129])
    nc.sync.dma_start(out=hfpad[64:112, 0:1, 1:129], in_=hfpad[0:48, 64:65, 1:129])
    tap_pad('hf', hfpad)

    # ============ P5: lowrev + U/V mixes ============
    lowwrev = a2c.tile([NP, HH, W], BF, tag="s3")
    nc.vector.tensor_copy(lowwrev[:, :, 0:1], low[:, :, 0:1])
    rev_src = _ap(low[...], W - 1, [[HH * W, NP], [W, HH], [-1, 127]])
    nc.vector.tensor_copy(lowwrev[:, :, 1:128], rev_src)
    lowrev = a2c.tile([NP, HH, W], BF, tag="s4")
    nc.sync.dma_start(out=lowrev[0:48, 0:1, :], in_=lowwrev[0:48, 0:1, :])
    nc.sync.dma_start(out=lowrev[64:112, 0:1, :], in_=lowwrev[64:112, 0:1, :])
    src1 = _ap(lowwrev[...], 64 * (HH * W) + 63 * W,
               [[HH * W, 48], [-W, 63], [1, W]])
    nc.sync.dma_start(out=lowrev[0:48, 1:64, :], in_=src1)
    src2 = _ap(lowwrev[...], 63 * W, [[HH * W, 48], [-W, 63], [1, W]])
    nc.sync.dma_start(out=lowrev[64:112, 1:64, :], in_=src2)

    # mixes stream through small staging tiles straight to the DRAM
    # bounce buffers ((h, c, w) order); Wre/Wim never materialize in SBUF.
    lowD = dram.tile([H, C, W], BF, tag="lowD")
    WreD = dram.tile([H, C, W], BF, tag="WreD")
    WimD = dram.tile([H, C, W], BF, tag="WimD")
    for dstD, u, v, spike in ((WreD, 'UrT', 'VrT', 'flbP'),
                              (WimD, 'UiT', 'ViT', 'flbQ')):
        for ch in range(16):
            rows = slice(4 * ch, 4 * ch + 4)
            pt = ps_mm.tile([NP, 512], F32, tag="mm")
            for p0, mo in ((0, 64), (64, 48)):
                nc.tensor.matmul(pt[p0:p0 + mo, :],
                                 cc[u][p0:p0 + 48, 0:mo],
                                 low[p0:p0 + 48, rows, :], start=True,
                                 stop=False, tile_position=(p0, p0),
                                 skip_group_check=True)
            for p0, mo in ((0, 64), (64, 48)):
                nc.tensor.matmul(pt[p0:p0 + mo, :],
                                 cc[v][p0:p0 + 48, 0:mo],
                                 lowrev[p0:p0 + 48, rows, :], start=False,
                                 stop=True, tile_position=(p0, p0),
                                 skip_group_check=True)
            wst = scr.tile([NP, 512], BF, tag="wst")
            evac(wst[...], pt[...], ch % 2)
            if ch == 0:
                nc.vector.tensor_scalar(wst[0:48, 0:1], wst[0:48, 0:1],
                                        cc[spike][...], None, OP.add)
            for hb, p0 in ((0, 0), (1, 64)):
                dst = _ap(dstD[...], (hb * HH + 4 * ch) * C * W,
                          [[W, 48], [C * W, 4], [1, W]])
                nc.sync.dma_start(out=dst, in_=_ap(wst[...], p0 * 512,
                                                   [[512, 48], [128, 4],
                                                    [1, 128]]))
    for hb, p0 in ((0, 0), (1, 64)):
        dst = _ap(lowD[...], hb * HH * C * W,
                  [[W, 48], [C * W, HH], [1, W]])
        nc.sync.dma_start(out=dst, in_=low[p0:p0 + 48, :, :])

    # ============ P7: FFT per 4-channel group ============
    lfD = dram.tile([C, H, W], BF, tag="lfD")

    def transpose_pair(src, out_tag):
        d = scrF.tile([128, 1024], BF, tag="fbig")
        for half in range(2):
            pt = ps_tr.tile([128, 512], BF, tag="tr")
            for c4 in range(4):
                nc.tensor.transpose(
                    pt[:, c4 * 128:(c4 + 1) * 128],
                    src[:, half * 512 + c4 * 128:half * 512 + (c4 + 1) * 128],
                    cc['I128b'][...])
            evac(d[:, half * 512:(half + 1) * 512], pt[...], half)
        return d

    for grp in range(12):
        c0 = grp * 4
        ld = {}
        for nm, dten in (('lo', lowD), ('wr', WreD), ('wi', WimD)):
            t = scrFi.tile([128, 512], BF, tag="fi")
            src = _ap(dten[...], c0 * W, [[C * W, 128], [W, 4], [1, W]])
            nc.sync.dma_start(out=t[...], in_=src)
            ld[nm] = t

        # W path: stage A (contract h), complex input
        sA = scrF.tile([128, 1024], BF, tag="fbig")
        pre = ps_tr.tile([128, 512], F32, tag="tr")
        pim = ps_tr.tile([128, 512], F32, tag="tr")
        nc.tensor.matmul(pre[...], cc['CH'][...], ld['wr'][...],
                         start=True, stop=False)
        nc.tensor.matmul(pim[...], cc['CH'][...], ld['wi'][...],
                         start=True, stop=False)
        nc.tensor.matmul(pre[...], cc['SH'][...], ld['wi'][...],
                         start=False, stop=True)
        nc.tensor.matmul(pim[...], cc['SHn'][...], ld['wr'][...],
                         start=False, stop=True)
        evac(sA[:, 0:512], pre[...], 0)
        evac(sA[:, 512:1024], pim[...], 1)
        sAT = transpose_pair(sA, "sAT")
        # stage B (contract w) -> gelu gates
        gre = ps_tr.tile([128, 512], F32, tag="tr")
        gim = ps_tr.tile([128, 512], F32, tag="tr")
        nc.tensor.matmul(gre[0:65, :], cc['CW'][...], sAT[:, 0:512],
                         start=True, stop=False)
        nc.tensor.matmul(gim[0:65, :], cc['CW'][...], sAT[:, 512:1024],
                         start=True, stop=False)
        nc.tensor.matmul(gre[0:65, :], cc['SW'][...], sAT[:, 512:1024],
                         start=False, stop=True)
        nc.tensor.matmul(gim[0:65, :], cc['SWn'][...], sAT[:, 0:512],
                         start=False, stop=True)
        gate = scr.tile([65, 1024], BF, tag="gz")
        nc.scalar.activation(gate[:, 0:512], gre[0:65, :], AF.Gelu)
        nc.scalar.activation(gate[:, 512:1024], gim[0:65, :], AF.Gelu)

        # low path
        sY = scrF.tile([128, 1024], BF, tag="fbig")
        pyr = ps_tr.tile([128, 512], F32, tag="tr")
        pyi = ps_tr.tile([128, 512], F32, tag="tr")
        nc.tensor.matmul(pyr[...], cc['CH'][...], ld['lo'][...],
                         start=True, stop=True)
        nc.tensor.matmul(pyi[...], cc['SHn'][...], ld['lo'][...],
                         start=True, stop=True)
        evac(sY[:, 0:512], pyr[...], 0)
        evac(sY[:, 512:1024], pyi[...], 1)
        sYT = transpose_pair(sY, "sYT")
        zre = ps_tr.tile([128, 512], F32, tag="tr")
        zim = ps_tr.tile([128, 512], F32, tag="tr")
        nc.tensor.matmul(zre[0:65, :], cc['CW'][...], sYT[:, 0:512],
                         start=True, stop=False)
        nc.tensor.matmul(zim[0:65, :], cc['CW'][...], sYT[:, 512:1024],
                         start=True, stop=False)
        nc.tensor.matmul(zre[0:65, :], cc['SW'][...], sYT[:, 512:1024],
                         start=False, stop=True)
        nc.tensor.matmul(zim[0:65, :], cc['SWn'][...], sYT[:, 0:512],
                         start=False, stop=True)
        zm = scr.tile([65, 1024], BF, tag="gz")
        nc.vector.tensor_mul(zm[:, 0:512], zre[0:65, :], gate[:, 0:512])
        nc.vector.tensor_mul(zm[:, 512:1024], zim[0:65, :],
                             gate[:, 512:1024])

        # inverse: contract fw then fh
        sAB = scrF.tile([128, 1024], BF, tag="fbig")
        am = ps_tr.tile([128, 512], F32, tag="tr")
        bm = ps_tr.tile([128, 512], F32, tag="tr")
        nc.tensor.matmul(am[...], cc['IWcT'][...], zm[:, 0:512],
                         start=True, stop=False)
        nc.tensor.matmul(bm[...], cc['IWsT'][...], zm[:, 0:512],
                         start=True, stop=False)
        nc.tensor.matmul(am[...], cc['IWsT'][...], zm[:, 512:1024],
                         start=False, stop=True)
        nc.tensor.matmul(bm[...], cc['IWcTn'][...], zm[:, 512:1024],
                         start=False, stop=True)
        evac(sAB[:, 0:512], am[...], 0)
        evac(sAB[:, 512:1024], bm[...], 1)
        sABT = transpose_pair(sAB, "sABT")
        xl = ps_tr.tile([128, 512], F32, tag="tr")
        nc.tensor.matmul(xl[...], cc['cH2'][...], sABT[:, 0:512],
                         start=True, stop=False)
        nc.tensor.matmul(xl[...], cc['sH2'][...], sABT[:, 512:1024],
                         start=False, stop=True)
        xst = scr.tile([128, 512], BF, tag="wst")
        nc.vector.tensor_copy(xst[...], xl[...])
        dst = _ap(lfD[...], c0 * HW, [[128, 128], [HW, 4], [1, 128]])
        nc.sync.dma_start(out=dst, in_=xst[...])

    lfpad = pads.tile([NP, 66, 130], BF, tag="p0")
    pad_border(lfpad)
    nc.sync.dma_start(out=lfpad[0:48, 1:66, 1:129], in_=lfD[:, 0:65, :])
    nc.sync.dma_start(out=lfpad[64:112, 0:65, 1:129], in_=lfD[:, 63:128, :])
    if 'lf' in tap_d:
        g.dma_start(out=tap_d['lf'][...], in_=lfD[...])

    # ============ P9/P10: attention ============
    def fga_core(q, k, v, pre, to, tname, wpt, tapn):
        qf = q[...].rearrange("p a b -> p (a b)")
        kf = k[...].rearrange("p a b -> p (a b)")
        # One folded gram G[128, 128]: lhsT = rhs = 128-col strided view of
        # qkt picking (q-half | k-half) with the A2 gaps as ignored junk.
        # G[0:48, 0:48] = qq (halves summed in psum), G[0:48, 64:112] = qk,
        # G[64:112, 64:112] = kk.  LDW is 128-wide -> FWL kicks in.
        G = ps_g.tile([128, 128], F32, tag="gA")
        qkts = {}
        for sc in range(65):
            if sc < 64:
                pt = ps_tr.tile([128, 224], BF, tag="tr")
                nc.tensor.transpose(pt[:, 0:112],
                                    qf[:, 128 * sc:128 * sc + 128],
                                    cc['I128b'][0:112, 0:112])
                nc.tensor.transpose(pt[:, 112:224],
                                    kf[:, 128 * sc:128 * sc + 128],
                                    cc['I128b'][0:112, 0:112])
                qkt_n = scr.tile([128, 256], BF, tag="qkt",
                                 name="qkt%d" % sc)
                # [q-h0 | k-h0 | q-h1 | k-h1] in 64-col blocks so the gram
                # lhsT/rhs are single contiguous 128-col slices (FWL-wide).
                which = sc % 2
                evac(qkt_n[:, 0:48], pt[:, 0:48], which)
                evac(qkt_n[:, 64:112], pt[:, 112:160], which)
                evac(qkt_n[:, 128:176], pt[:, 64:112], 1 - which)
                evac(qkt_n[:, 192:240], pt[:, 176:224], 1 - which)
                qkts[sc] = qkt_n
            if sc == 0:
                continue
            gc = sc - 1
            qkt = qkts.pop(gc)
            for hb in (0, 1):
                ap = qkt[:, 128 * hb:128 * hb + 128]
                nc.tensor.matmul(G[...], ap, ap,
                                 start=(gc == 0 and hb == 0),
                                 stop=(gc == 63 and hb == 1),
                                 skip_group_check=True)

        t48 = sm.tile([48, 48], F32, tag="t48")
        dq = sm.tile([48, 1], F32, tag="dq")
        tk48 = sm.tile([NP, 48], F32, tag="tk48")
        dkv = sm.tile([NP, 1], F32, tag="dkv")
        nc.vector.tensor_mul(t48[...], G[0:48, 0:48], cc['Ia2f'][0:48, :])
        nc.vector.reduce_sum(dq[...], t48[...], AX.X)
        nc.vector.tensor_mul(tk48[64:112, :], G[64:112, 64:112],
                             cc['Ia2f'][64:112, :])
        nc.vector.reduce_sum(dkv[64:112, :], tk48[64:112, :], AX.X)
        nc.scalar.activation(dq[...], dq[...], AF.Sqrt)
        nc.vector.reciprocal(dq[...], dq[...])
        nc.scalar.activation(dkv[64:112, :], dkv[64:112, :], AF.Sqrt)
        nc.vector.reciprocal(dkv[64:112, :], dkv[64:112, :])
        nc.vector.tensor_mul(dq[...], dq[...], cc[tname][...])
        prk = ps_g.tile([1, 48], F32, tag="gsm")
        nc.tensor.matmul(prk[...], dkv[64:112, :], cc['Ia2f'][64:112, :],
                         start=True, stop=True)
        rkrow = sm.tile([1, 48], F32, tag="rkrow")
        nc.vector.tensor_copy(rkrow[...], prk[...])
        rkmat = sm.tile([48, 48], F32, tag="rkmat")
        nc.gpsimd.partition_broadcast(rkmat[...], rkrow[...])

        S = sm.tile([48, 48], F32, tag="S")
        nc.vector.scalar_tensor_tensor(S[...], G[0:48, 64:112], dq[...],
                                       rkmat[...], OP.mult, OP.mult)
        nc.vector.tensor_add(S[...], S[...], cc['mask48'][...])
        nmax = sm.tile([48, 1], F32, tag="nmax")
        nc.vector.reduce_max(nmax[...], S[...], AX.X, negate=True)
        nc.scalar.activation(S[...], S[...], AF.Exp, bias=nmax[...])
        ssum = sm.tile([48, 1], F32, tag="ssum")
        nc.vector.reduce_sum(ssum[...], S[...], AX.X)
        nc.vector.reciprocal(ssum[...], ssum[...])
        nc.vector.tensor_scalar(S[...], S[...], ssum[...], None, OP.mult)
        pm2 = ps_g.tile([48, 48], F32, tag="gsm")
        nc.tensor.matmul(pm2[...], S[...], cc[wpt][...], start=True,
                         stop=True)
        m2 = sm.tile([NP, 64], BF, tag="m2")
        nc.vector.memset(m2[0:48, 48:64], 0.0)
        nc.vector.tensor_copy(m2[0:48, 0:48], pm2[...])
        nc.sync.dma_start(out=m2[64:112, 0:48], in_=m2[0:48, 0:48])

        o = a2c.tile([NP, HH, W], BF, tag=to)
        for ch in range(16):
            pt = ps_mm.tile([NP, 512], F32, tag="mm")
            for p0, mo in ((0, 64), (64, 48)):
                nc.tensor.matmul(pt[p0:p0 + mo, :], m2[p0:p0 + 48, 0:mo],
                                 v[p0:p0 + 48, 4 * ch:4 * ch + 4, :],
                                 start=True, stop=True,
                                 tile_position=(p0, p0),
                                 skip_group_check=True)
            evac(o[:, 4 * ch:4 * ch + 4, :], pt[...], ch % 2)
        tap_a2(tapn, o)
        return o

    qh = a2c.tile([NP, HH, W], BF, tag="s0")
    conv_taps(qh, hfpad, cc['hqT'])
    ho = fga_core(qh, kh, vh, 'h', "s3", 't_h', 'WpT_h', 'ho')
    ql = a2c.tile([NP, HH, W], BF, tag="s0")
    kl = a2c.tile([NP, HH, W], BF, tag="s1")
    vl = a2c.tile([NP, HH, W], BF, tag="s2")
    conv_taps(ql, lfpad, cc['lqT'])
    conv_taps(kl, xpad0, cc['lkT'])
    conv_taps(vl, xpad0, cc['lvT'])
    lo = fga_core(ql, kl, vl, 'l', "s4", 't_l', 'WpT_l', 'lo')

    # ============ P11: final projection + bias + residual ============
    for ch in range(16):
        rows = slice(4 * ch, 4 * ch + 4)
        pt = ps_mm.tile([NP, 512], F32, tag="mm")
        for p0, mo in ((0, 64), (64, 48)):
            nc.tensor.matmul(pt[p0:p0 + mo, :],
                             cc['fpw_hT'][p0:p0 + 48, 0:mo],
                             ho[p0:p0 + 48, rows, :], start=True, stop=False,
                             tile_position=(p0, p0), skip_group_check=True)
        for p0, mo in ((0, 64), (64, 48)):
            nc.tensor.matmul(pt[p0:p0 + mo, :],
                             cc['fpw_lT'][p0:p0 + 48, 0:mo],
                             lo[p0:p0 + 48, rows, :], start=False, stop=True,
                             tile_position=(p0, p0), skip_group_check=True)
        ost = scr.tile([NP, 512], F32, tag="ost")
        nc.vector.scalar_tensor_tensor(
            ost[...].rearrange("p (a b) -> p a b", a=4),
            pt[...].rearrange("p (a b) -> p a b", a=4),
            cc['fp_bv'][...],
            xpad0[:, 1 + 4 * ch:5 + 4 * ch, 1:129], OP.add, OP.add)
        nc.sync.dma_start(out=out_d[:, 4 * ch:4 * ch + 4, :],
                          in_=ost[0:48, :])
        nc.sync.dma_start(out=out_d[:, 64 + 4 * ch:64 + 4 * ch + 4, :],
                          in_=ost[64:112, :])


# ======================= host-side runner =======================
_CACHE = {}


def get_module(taps=()):
    key = tuple(sorted(taps))
    if key not in _CACHE:
        _CACHE[key] = build_module(taps)
    return _CACHE[key]


def make_in_maps(inputs, n_cores=8):
    """inputs: full dict per setup_inputs(). Returns per-core in_maps."""
    w = {k: np.asarray(v, np.float32) for k, v in inputs.items()
         if k != 'x'}
    cst = host_consts(w)
    cst_cast = {}
    for k, v in cst.items():
        cst_cast[k] = np.ascontiguousarray(v)
    x = np.asarray(inputs['x'], np.float32)
    maps = []
    for core in range(n_cores):
        m = dict(cst_cast)
        m['x'] = np.ascontiguousarray(x[core])
        maps.append(m)
    return maps


def kernel(**inputs):
    """Full-input entry: shards batch over 8 NeuronCores, returns full output.

    inputs: exactly the dict produced by setup_inputs() (numpy arrays).
    """
    from concourse.bass_utils import run_bass_kernel_spmd
    x = np.asarray(inputs['x'], np.float32)
    n_cores = x.shape[0]
    nc = get_module(())
    in_maps = make_in_maps(inputs, n_cores=n_cores)
    res = run_bass_kernel_spmd(nc, in_maps, core_ids=list(range(n_cores)),
                               trace=False)
    out = np.stack([res.results[c]["out"] for c in range(n_cores)])
    return out.astype(np.float32)



# revision 24
# speedup vs baseline: 1.0291x; 1.0291x over previous
"""Bass/Tile kernel for nn_FMMPreWork on TRN2, one sample per NeuronCore.

Layouts
-------
A2: channel-major tensors on 112 partitions:
  p in [0,48):   channel c=p,    spatial half 0 (h in [0,64))
  p in [64,112): channel c=p-64, spatial half 1 (h in [64,128))
  partitions [48,64) are a gap (enables 2-way PE row+col packing).
Padded A2 tile [112, 66, 130]: interior [:, 1:65, 1:129]; local row r is
  h = r-1 (half0) / h = 63+r (half1); col j is w = j-1.
Compact A2 tile [112, 64, 128].
FFT runs per 4-channel group in [spatial, (ch, spatial)] layouts staged
through DRAM bounce buffers ((h, c, w)-ordered).

The frequency-domain channel mix (M yf + b) is moved to the spatial domain:
  M yf + b = FFT2(U low + V lowrev + b delta_00),  U/V complex [48,48].
Attention uses gram-folded normalization: q/k L2 norms come from the same
PSUM-accumulated gram products, so q/k are never explicitly normalized.
"""
import math
from itertools import product

import ml_dtypes
import numpy as np

import concourse.bass as bass
import concourse.mybir as mybir
import concourse.tile as tile
from concourse import bacc

DT = mybir.dt
BF = DT.bfloat16
FR = DT.float32r
F32 = DT.float32
AF = mybir.ActivationFunctionType
OP = mybir.AluOpType
AX = mybir.AxisListType

C = 48
H = W = 128
HH = 64
HW = H * W
NP = 112
NF = 65
GROUP = 8
HEADS = 4
BN_EPS = 1e-5
MASKVAL = -60.0

bf16 = ml_dtypes.bfloat16

_WSHAPES = {
    'fd_w': (72, 48), 'bn_g': (72,), 'bn_b': (72,), 'bn_m': (72,),
    'bn_v': (72,), 'fh_w1': (48, 1, 1, 3), 'fh_b1': (48,),
    'fh_w2': (48, 1, 3, 1), 'fh_b2': (48,), 'fl_w': (96, 96), 'fl_b': (96,),
    'hfa_qw': (48, 48), 'hfa_kvw': (96, 48), 'hfa_qdw': (48, 1, 3, 3),
    'hfa_kvdw': (96, 1, 3, 3), 'hfa_pw': (48, 48), 'hfa_t': (4, 1, 1),
    'lfa_qw': (48, 48), 'lfa_kvw': (96, 48), 'lfa_qdw': (48, 1, 3, 3),
    'lfa_kvdw': (96, 1, 3, 3), 'lfa_pw': (48, 48), 'lfa_t': (4, 1, 1),
    'fp_w': (48, 96), 'fp_b': (48,),
}


def a2vec(v48, dtype=np.float32):
    out = np.zeros((NP, 1), dtype=dtype)
    out[0:48, 0] = v48
    out[64:112, 0] = v48
    return out


def a2mat(m, dtype):
    x = np.asarray(m)
    out = np.zeros((NP,) + x.shape[1:], np.float64)
    out[0:48] = x
    out[64:112] = x
    return out.astype(dtype)


def a2mat64(m, dtype):
    """[48, 48] lhsT -> [112, 64] with zero-padded out-channels 48:64 so
    matmuls at tile (0,0) cover psum partitions 0..64 (gap stays zeroed)."""
    x = np.asarray(m)
    out = np.zeros((NP, 64), np.float64)
    out[0:48, 0:48] = x
    out[64:112, 0:48] = x
    return out.astype(dtype)


def host_consts(w):
    """Derived constant inputs. w: dict of np.float32 weight arrays."""
    cst = {}
    f8 = np.float64

    s = 1.0 / np.sqrt(w['bn_v'].astype(f8) + BN_EPS)
    fdw_f = (w['bn_g'].astype(f8) * s)[:, None] * w['fd_w'].astype(f8)
    fdb_f = w['bn_b'].astype(f8) - w['bn_m'].astype(f8) * w['bn_g'].astype(f8) * s
    cst['fdw_fT'] = np.ascontiguousarray(fdw_f.T).astype(np.float32)
    cst['fdb_f'] = fdb_f.astype(np.float32).reshape(72, 1)
    ps = np.zeros((NP, 48), np.float32)
    for c in range(48):
        ps[c, c] = 1.0 / HW
        ps[64 + c, c] = 1.0 / HW
    cst['pairsum'] = ps
    cst['I72f'] = np.eye(72, dtype=np.float32)
    cst['I48f'] = np.eye(48, dtype=np.float32)
    cst['I128b'] = np.eye(128, dtype=np.float32).astype(bf16)

    cst['fh_w1v'] = np.concatenate(
        [a2vec(w['fh_w1'][:, 0, 0, j]) for j in range(3)], axis=1)
    cst['fh_w2v'] = np.concatenate(
        [a2vec(w['fh_w2'][:, 0, i, 0]) for i in range(3)], axis=1)
    cst['fh_b1v'] = a2vec(w['fh_b1'])
    cst['fh_b2v'] = a2vec(w['fh_b2'])

    def fuse(w1, dw, rows):
        t = np.zeros((NP, 9, 64), np.float64)
        for i in range(3):
            for j in range(3):
                Wd = dw[rows, 0, i, j].astype(f8)[:, None] * w1[rows, :].astype(f8)
                t[0:48, i * 3 + j, 0:48] = Wd.T
                t[64:112, i * 3 + j, 0:48] = Wd.T
        return t.astype(bf16)
    for pre, qw, kvw, qdw, kvdw in (
            ('h', w['hfa_qw'], w['hfa_kvw'], w['hfa_qdw'], w['hfa_kvdw']),
            ('l', w['lfa_qw'], w['lfa_kvw'], w['lfa_qdw'], w['lfa_kvdw'])):
        cst[pre + 'qT'] = fuse(qw, qdw, slice(0, 48))
        cst[pre + 'kT'] = fuse(kvw, kvdw, slice(0, 48))
        cst[pre + 'vT'] = fuse(kvw, kvdw, slice(48, 96))

    A = w['fl_w'][0:C, 0:C].astype(f8)
    B = w['fl_w'][0:C, C:2 * C].astype(f8)
    Cb = w['fl_w'][C:2 * C, 0:C].astype(f8)
    D = w['fl_w'][C:2 * C, C:2 * C].astype(f8)
    for name, M in (('UrT', (A + D) / 2), ('UiT', (Cb - B) / 2),
                    ('VrT', (A - D) / 2), ('ViT', (Cb + B) / 2)):
        cst[name] = a2mat64(M.T, bf16)
    cst['flbP'] = w['fl_b'][0:C].astype(np.float32).reshape(48, 1)
    cst['flbQ'] = w['fl_b'][C:2 * C].astype(np.float32).reshape(48, 1)

    n = np.arange(128)
    th = 2 * np.pi * np.outer(n, n) / 128.0
    thw = 2 * np.pi * np.outer(n, np.arange(NF)) / 128.0
    CH = np.cos(th); SH = np.sin(th)
    CW = np.cos(thw); SW = np.sin(thw)
    cst['CH'] = CH.astype(bf16); cst['SH'] = SH.astype(bf16)
    cst['SHn'] = (-SH).astype(bf16)
    cst['CW'] = CW.astype(bf16); cst['SW'] = SW.astype(bf16)
    cst['SWn'] = (-SW).astype(bf16)
    ck = np.ones(NF); ck[1:NF - 1] = 2.0
    IWc = (ck[None, :] / 128.0) * np.cos(thw)
    IWs = -(ck[None, :] / 128.0) * np.sin(thw)
    cst['IWcT'] = np.ascontiguousarray(IWc.T).astype(bf16)
    cst['IWsT'] = np.ascontiguousarray(IWs.T).astype(bf16)
    cst['IWcTn'] = np.ascontiguousarray(-IWc.T).astype(bf16)
    cst['cH2'] = (CH / 128.0).astype(bf16)
    cst['sH2'] = (SH / 128.0).astype(bf16)

    cst['Ia2'] = a2mat64(np.eye(48), bf16)
    d1 = np.zeros((NP, 3, 64), np.float64)
    d2 = np.zeros((NP, 3, 64), np.float64)
    for jj in range(3):
        d1[0:48, jj, 0:48] = np.diag(w['fh_w1'][:, 0, 0, jj].astype(f8))
        d1[64:112, jj, 0:48] = d1[0:48, jj, 0:48]
        d2[0:48, jj, 0:48] = np.diag(w['fh_w2'][:, 0, jj, 0].astype(f8))
        d2[64:112, jj, 0:48] = d2[0:48, jj, 0:48]
    cst['fhD1'] = d1.astype(bf16)
    cst['fhD2'] = d2.astype(bf16)
    cst['ones11'] = np.ones((1, 1), np.float32)
    t9 = np.zeros((72, 9), np.float32)
    for j in range(72):
        t9[j, j % 9] = 1.0
    cst['T9'] = t9
    selc = np.zeros((72, NP), np.float32)
    for c in range(48):
        for j in range(72):
            if j // 9 == c // 6:
                selc[j, c] = 1.0
                selc[j, 64 + c] = 1.0
    cst['SelC'] = selc

    cst['Ia2f'] = a2mat(np.eye(48), np.float32)
    hm = np.arange(48) // (C // HEADS)
    cst['mask48'] = np.where(hm[:, None] == hm[None, :], 0.0,
                             MASKVAL).astype(np.float32)
    cst['t_h'] = w['hfa_t'][hm, 0, 0].astype(np.float32).reshape(48, 1)
    cst['t_l'] = w['lfa_t'][hm, 0, 0].astype(np.float32).reshape(48, 1)
    cst['WpT_h'] = np.ascontiguousarray(w['hfa_pw'].T).astype(np.float32)
    cst['WpT_l'] = np.ascontiguousarray(w['lfa_pw'].T).astype(np.float32)

    cst['fpw_hT'] = a2mat64(w['fp_w'][:, 0:C].astype(f8).T, bf16)
    cst['fpw_lT'] = a2mat64(w['fp_w'][:, C:2 * C].astype(f8).T, bf16)
    cst['fp_bv'] = a2vec(w['fp_b'])
    return cst


def _ap(base, offset_delta, dims):
    return bass.AP(tensor=base.tensor, offset=base.offset + offset_delta,
                   ap=dims)


def build_module(taps=()):
    nc = bacc.Bacc('TRN2', target_bir_lowering=False, debug=False,
                   num_devices=8)
    x_d = nc.dram_tensor("x", [C, H, W], F32, kind="ExternalInput")
    out_d = nc.dram_tensor("out", [C, H, W], F32, kind="ExternalOutput")

    shapes = host_consts({k: np.zeros(s, np.float32)
                          for k, s in _WSHAPES.items()})
    cst_d = {}
    for k, v in shapes.items():
        dt = BF if v.dtype == bf16 else F32
        cst_d[k] = nc.dram_tensor(k, list(v.shape), dt, kind="ExternalInput")

    tap_d = {}
    for t in taps:
        shape = {'w72': [1, 72]}.get(t, [C, H, W])
        tap_d[t] = nc.dram_tensor("tap_" + t, shape, F32,
                                  kind="ExternalOutput")

    with tile.TileContext(nc) as tc:
        _build(nc, tc, x_d, out_d, cst_d, tap_d)
    nc.compile()
    return nc


def _build(nc, tc, x_d, out_d, cst_d, tap_d):
    import contextlib
    with contextlib.ExitStack() as ctx:
        ep = ctx.enter_context
        cp = ep(tc.tile_pool(name="consts", bufs=1))
        dram = ep(tc.tile_pool(name="dram", bufs=1, space="DRAM"))
        pads = ep(tc.tile_pool(name="pads", bufs=1))
        a2c = ep(tc.tile_pool(name="a2c", bufs=1))
        scr = ep(tc.tile_pool(name="scr", bufs=2))
        scrF = ep(tc.tile_pool(name="scrF", bufs=4))
        scrFi = ep(tc.tile_pool(name="scrFi", bufs=3))
        sm = ep(tc.tile_pool(name="sm", bufs=1))
        ps_mm = ep(tc.tile_pool(name="psmm", bufs=3, space="PSUM"))
        ps_tr = ep(tc.tile_pool(name="pstr", bufs=3, space="PSUM"))
        ps_g = ep(tc.tile_pool(name="psg", bufs=1, space="PSUM"))
        _body(nc, x_d, out_d, cst_d, tap_d,
              cp, dram, pads, a2c, scr, scrF, scrFi, sm, ps_mm, ps_tr, ps_g)


def _body(nc, x_d, out_d, cst_d, tap_d,
          cp, dram, pads, a2c, scr, scrF, scrFi, sm, ps_mm, ps_tr, ps_g):
    g = nc.gpsimd

    # ============ P0: raw x in SBUF (A2 halves with 1-row overlap) ==========
    # xraw[0:48, r, :]  = x[c, r, :]      for r in [0, 65)   (h = 0..64)
    # xraw[64:112, r, :] = x[c, 63+r, :]  for r in [0, 65)   (h = 63..127)
    # Time-shares the hfpad buffer (tag p2): every xraw read happens long
    # before hfpad is written.  Flat per-partition layout keeps the input
    # DMA a single contiguous 16.6 KB run per partition.
    XPITCH = 66 * 130
    xrawT = pads.tile([NP, 66, 130], BF, tag="p2")

    def xrv(ps, rs, cs=slice(0, W)):
        return _ap(xrawT[...],
                   ps.start * XPITCH + rs.start * W + cs.start,
                   [[XPITCH, ps.stop - ps.start], [W, rs.stop - rs.start],
                    [1, cs.stop - cs.start]])

    g.dma_start(out=xrv(slice(0, 48), slice(0, 65)), in_=x_d[:, 0:65, :])
    g.dma_start(out=xrv(slice(64, 112), slice(0, 65)), in_=x_d[:, 63:128, :])

    cc = {}
    order = ['hkT', 'hvT', 'pairsum', 'fdw_fT', 'fdb_f', 'I72f', 'ones11',
             'T9', 'SelC', 'Ia2', 'fh_b1v', 'fh_b2v', 'fhD1', 'fhD2',
             'I128b', 'I48f']
    keys = order + [k for k in cst_d if k not in order]
    qs = [nc.sync, nc.scalar, nc.gpsimd]
    for i, k in enumerate(keys):
        d = cst_d[k]
        t = cp.tile(list(d.shape), d.dtype, tag="c_" + k)
        qs[i % 3].dma_start(out=t[...], in_=d[...])
        cc[k] = t

    def tap_a2(name, src):
        """compact A2 [112, 64, 128] -> tap [C, H, W] f32"""
        if name not in tap_d:
            return
        d = tap_d[name]
        for hb, p0 in ((0, 0), (1, 64)):
            g.dma_start(out=d[:, hb * HH:(hb + 1) * HH, :],
                        in_=src[p0:p0 + 48, :, :])

    def tap_pad(name, src):
        if name not in tap_d:
            return
        d = tap_d[name]
        for hb, p0 in ((0, 0), (1, 64)):
            g.dma_start(out=d[:, hb * HH:(hb + 1) * HH, :],
                        in_=src[p0:p0 + 48, 1:65, 1:129])

    def evac(dst_ap, src_ap, which):
        if which == 0:
            nc.vector.tensor_copy(dst_ap, src_ap)
        else:
            nc.scalar.copy(dst_ap, src_ap)

    def tap_conv(srcpad, tapsT, taps, evac_fn):
        """taps: list of (t_idx, dh, dw); evac_fn(ch, psum)."""
        nt = len(taps)
        for ch in range(16):
            pt = ps_mm.tile([NP, 512], F32, tag="mm")
            for n, (t, i, j) in enumerate(taps):
                for p0, mo in ((0, 64), (64, 48)):
                    rhs = srcpad[p0:p0 + 48, 4 * ch + i:4 * ch + i + 4,
                                 j:j + 128]
                    nc.tensor.matmul(pt[p0:p0 + mo, :],
                                     tapsT[p0:p0 + 48, t, 0:mo], rhs,
                                     start=(n == 0), stop=(n == nt - 1),
                                     tile_position=(p0, p0),
                                     skip_group_check=True)
            evac_fn(ch, pt)

    TAPS9 = [(t, i, j) for t, (i, j) in
             enumerate(product(range(3), range(3)))]

    def conv_taps(dst, srcpad, tapsT):
        tap_conv(srcpad, tapsT, TAPS9,
                 lambda ch, pt: evac(dst[:, 4 * ch:4 * ch + 4, :], pt[...],
                                     ch % 2))


    def pad_border(t):
        """Zero borders only. Gap partitions [48:64] are left unwritten:
        conv rhs reads use [0:48]/[64:112] slices and never touch them."""
        nc.vector.memset(t[0:48, 0:1, :], 0.0)
        nc.vector.memset(t[64:112, 65:66, :], 0.0)
        g.memset(t[:, :, 0:1], 0.0)
        g.memset(t[:, :, 129:130], 0.0)

    # ============ P1: padded x (built on-chip from xraw) ============
    xpadR = pads.tile([NP, 66, 130], BF, tag="p0")
    xpad0 = pads.tile([NP, 66, 130], BF, tag="px")
    h0, h1 = slice(0, 48), slice(64, 112)
    # reflect pad, half0 (local rows r=0..65 are h=-1..64)
    nc.vector.tensor_copy(xpadR[0:48, 1:66, 1:129], xrv(h0, slice(0, 65)))
    nc.scalar.copy(xpadR[64:112, 0:65, 1:129], xrv(h1, slice(0, 65)))
    nc.vector.tensor_copy(xpadR[0:48, 0:1, 1:129], xrv(h0, slice(1, 2)))
    nc.vector.tensor_copy(xpadR[64:112, 65:66, 1:129],
                          xrv(h1, slice(63, 64)))
    g.tensor_copy(xpadR[0:48, 1:66, 0:1],
                  xrv(h0, slice(0, 65), slice(1, 2)))
    g.tensor_copy(xpadR[0:48, 1:66, 129:130],
                  xrv(h0, slice(0, 65), slice(126, 127)))
    g.tensor_copy(xpadR[64:112, 0:65, 0:1],
                  xrv(h1, slice(0, 65), slice(1, 2)))
    g.tensor_copy(xpadR[64:112, 0:65, 129:130],
                  xrv(h1, slice(0, 65), slice(126, 127)))
    g.tensor_copy(xpadR[0:48, 0:1, 0:1], xrv(h0, slice(1, 2), slice(1, 2)))
    g.tensor_copy(xpadR[0:48, 0:1, 129:130],
                  xrv(h0, slice(1, 2), slice(126, 127)))
    g.tensor_copy(xpadR[64:112, 65:66, 0:1],
                  xrv(h1, slice(63, 64), slice(1, 2)))
    g.tensor_copy(xpadR[64:112, 65:66, 129:130],
                  xrv(h1, slice(63, 64), slice(126, 127)))
    # zero pad
    pad_border(xpad0)
    nc.vector.tensor_copy(xpad0[0:48, 1:66, 1:129], xrv(h0, slice(0, 65)))
    nc.scalar.copy(xpad0[64:112, 0:65, 1:129], xrv(h1, slice(0, 65)))

    # ============ P1.5: hfa k/v convs (only need xpad0; fill PE early) ====
    kh = a2c.tile([NP, HH, W], BF, tag="s1")
    vh = a2c.tile([NP, HH, W], BF, tag="s5")
    conv_taps(kh, xpad0, cc['hkT'])
    conv_taps(vh, xpad0, cc['hvT'])

    # ============ P2: mean -> softmax -> wvec ============
    part = sm.tile([NP, 1], F32, tag="part")
    nc.vector.memset(part[32:64, :], 0.0)
    nc.vector.tensor_reduce(part[0:48, :], xrv(h0, slice(0, 64)), AX.XY,
                            OP.add)
    nc.vector.tensor_reduce(part[64:112, :], xrv(h1, slice(1, 65)), AX.XY,
                            OP.add)
    pmean = ps_g.tile([48, 72], F32, tag="gsm")
    nc.tensor.matmul(pmean[:, 0:1], cc['pairsum'][...], part[...],
                     start=True, stop=True)
    mean48 = sm.tile([48, 1], F32, tag="mean48")
    nc.vector.tensor_copy(mean48[...], pmean[:, 0:1])
    pfd = ps_g.tile([72, 72], F32, tag="gsm")
    nc.tensor.matmul(pfd[:, 0:1], cc['fdw_fT'][...], mean48[...],
                     start=True, stop=True)
    smin = sm.tile([72, 1], F32, tag="smin")
    nc.vector.tensor_scalar(smin[...], pfd[:, 0:1], cc['fdb_f'][...], None,
                            OP.add)
    ptr72 = ps_g.tile([1, 72], F32, tag="gsm")
    nc.tensor.matmul(ptr72[...], smin[...], cc['I72f'][...],
                     start=True, stop=True)
    s72 = sm.tile([1, 72], F32, tag="s72")
    nc.vector.tensor_copy(s72[...], ptr72[...])
    s72v = s72[...].rearrange("p (g k) -> p g k", g=GROUP)
    mx8 = sm.tile([1, 8], F32, tag="mx8")
    nc.vector.reduce_max(mx8[...], s72v, AX.X)
    e72 = sm.tile([1, 72], F32, tag="e72")
    mxb = _ap(mx8[...], 0, [list(mx8[...].ap[0]), [1, 8], [0, 9]])
    nc.vector.tensor_sub(e72[...].rearrange("p (g k) -> p g k", g=GROUP),
                         s72v, mxb)
    nc.scalar.activation(e72[...], e72[...], AF.Exp)
    sum8 = sm.tile([1, 8], F32, tag="sum8")
    nc.vector.reduce_sum(sum8[...],
                         e72[...].rearrange("p (g k) -> p g k", g=GROUP),
                         AX.X)
    nc.vector.reciprocal(sum8[...], sum8[...])
    w72 = sm.tile([1, 72], F32, tag="w72")
    sumb = _ap(sum8[...], 0, [list(sum8[...].ap[0]), [1, 8], [0, 9]])
    nc.vector.tensor_mul(w72[...].rearrange("p (g k) -> p g k", g=GROUP),
                         e72[...].rearrange("p (g k) -> p g k", g=GROUP),
                         sumb)
    if 'w72' in tap_d:
        nc.sync.dma_start(out=tap_d['w72'][...], in_=w72[...])
    pw72c = ps_g.tile([72, 9], F32, tag="gsm")
    nc.tensor.matmul(pw72c[:, 0:1], w72[...], cc['ones11'][...],
                     start=True, stop=True)
    w72col = sm.tile([72, 1], F32, tag="w72col")
    nc.vector.tensor_copy(w72col[...], pw72c[:, 0:1])
    rhs9 = sm.tile([72, 9], F32, tag="rhs9")
    nc.vector.tensor_mul(rhs9[...], cc['T9'][...],
                         _ap(w72col[...], 0, [list(w72col[...].ap[0]),
                                              [0, 9]]))
    pwv = ps_g.tile([NP, 9], F32, tag="gsm")
    nc.tensor.matmul(pwv[...], cc['SelC'][...], rhs9[...],
                     start=True, stop=True)
    wvec = sm.tile([NP, 9], F32, tag="wvec")
    nc.vector.tensor_copy(wvec[...], pwv[...])

    # ============ P3: involution (depthwise -> DVE) + high ============
    # low[c, r, w] = sum_t wvec[c, t] * xpadR[c, r+i, w+j], t=(i, j).
    # Per-partition scalar multiply-accumulate; frees the PE entirely.
    low = a2c.tile([NP, HH, W], BF, tag="s2")
    nc.vector.tensor_scalar(low[...], xpadR[:, 0:64, 0:128], wvec[:, 0:1],
                            None, OP.mult)
    for t, (i, j) in enumerate(product(range(3), range(3))):
        if t == 0:
            continue
        nc.vector.scalar_tensor_tensor(low[...],
                                       xpadR[:, i:i + 64, j:j + 128],
                                       wvec[:, t:t + 1], low[...],
                                       OP.mult, OP.add)
    tap_a2('low', low)

    highpad = pads.tile([NP, 66, 130], BF, tag="p1")
    pad_border(highpad)
    nc.vector.scalar_tensor_tensor(
        highpad[0:48, 1:65, 1:129], low[0:48, :, :], -1.0,
        xrv(h0, slice(0, 64)), OP.mult, OP.add)
    nc.vector.scalar_tensor_tensor(
        highpad[64:112, 1:65, 1:129], low[64:112, :, :], -1.0,
        xrv(h1, slice(1, 65)), OP.mult, OP.add)
    nc.sync.dma_start(out=highpad[0:48, 65:66, 1:129],
                in_=highpad[64:112, 1:2, 1:129])
    nc.sync.dma_start(out=highpad[64:112, 0:1, 1:129],
                in_=highpad[0:48, 64:65, 1:129])
    tap_pad('high', highpad)

    # ============ P5: lowrev + U/V mixes ============
    lowwrev = a2c.tile([NP, HH, W], BF, tag="s3")
    nc.vector.tensor_copy(lowwrev[:, :, 0:1], low[:, :, 0:1])
    rev_src = _ap(low[...], W - 1, [[HH * W, NP], [W, HH], [-1, 127]])
    nc.vector.tensor_copy(lowwrev[:, :, 1:128], rev_src)
    lowrev = a2c.tile([NP, HH, W], BF, tag="s4")
    nc.sync.dma_start(out=lowrev[0:48, 0:1, :], in_=lowwrev[0:48, 0:1, :])
    nc.sync.dma_start(out=lowrev[64:112, 0:1, :], in_=lowwrev[64:112, 0:1, :])
    src1 = _ap(lowwrev[...], 64 * (HH * W) + 63 * W,
               [[HH * W, 48], [-W, 63], [1, W]])
    nc.sync.dma_start(out=lowrev[0:48, 1:64, :], in_=src1)
    src2 = _ap(lowwrev[...], 63 * W, [[HH * W, 48], [-W, 63], [1, W]])
    nc.sync.dma_start(out=lowrev[64:112, 1:64, :], in_=src2)

    # mixes stream through small staging tiles straight to the DRAM
    # bounce buffers ((h, c, w) order); Wre/Wim never materialize in SBUF.
    lowD = dram.tile([H, C, W], BF, tag="lowD")
    WreD = dram.tile([H, C, W], BF, tag="WreD")
    WimD = dram.tile([H, C, W], BF, tag="WimD")
    for dstD, u, v, spike in ((WreD, 'UrT', 'VrT', 'flbP'),
                              (WimD, 'UiT', 'ViT', 'flbQ')):
        for ch in range(16):
            rows = slice(4 * ch, 4 * ch + 4)
            pt = ps_mm.tile([NP, 512], F32, tag="mm")
            for p0, mo in ((0, 64), (64, 48)):
                nc.tensor.matmul(pt[p0:p0 + mo, :],
                                 cc[u][p0:p0 + 48, 0:mo],
                                 low[p0:p0 + 48, rows, :], start=True,
                                 stop=False, tile_position=(p0, p0),
                                 skip_group_check=True)
            for p0, mo in ((0, 64), (64, 48)):
                nc.tensor.matmul(pt[p0:p0 + mo, :],
                                 cc[v][p0:p0 + 48, 0:mo],
                                 lowrev[p0:p0 + 48, rows, :], start=False,
                                 stop=True, tile_position=(p0, p0),
                                 skip_group_check=True)
            wst = scr.tile([NP, 512], BF, tag="wst")
            evac(wst[...], pt[...], ch % 2)
            if ch == 0:
                nc.vector.tensor_scalar(wst[0:48, 0:1], wst[0:48, 0:1],
                                        cc[spike][...], None, OP.add)
            for hb, p0 in ((0, 0), (1, 64)):
                dst = _ap(dstD[...], (hb * HH + 4 * ch) * C * W,
                          [[W, 48], [C * W, 4], [1, W]])
                nc.sync.dma_start(out=dst, in_=_ap(wst[...], p0 * 512,
                                                   [[512, 48], [128, 4],
                                                    [1, 128]]))
    for hb, p0 in ((0, 0), (1, 64)):
        dst = _ap(lowD[...], hb * HH * C * W,
                  [[W, 48], [C * W, HH], [1, W]])
        nc.sync.dma_start(out=dst, in_=low[p0:p0 + 48, :, :])

    # ============ P4: fmgm_high ============
    y1pad = pads.tile([NP, 66, 130], BF, tag="p0")
    pad_border(y1pad)
    def evac_y1(ch, pt):
        nc.vector.tensor_scalar(y1pad[:, 1 + 4 * ch:1 + 4 * ch + 4, 1:129],
                                pt[...], cc['fh_b1v'][...], None, OP.add)
    tap_conv(highpad, cc['fhD1'], [(jj, 1, jj) for jj in range(3)], evac_y1)
    nc.sync.dma_start(out=y1pad[0:48, 65:66, 1:129], in_=y1pad[64:112, 1:2, 1:129])
    nc.sync.dma_start(out=y1pad[64:112, 0:1, 1:129], in_=y1pad[0:48, 64:65, 1:129])

    hfpad = pads.tile([NP, 66, 130], BF, tag="p2")
    pad_border(hfpad)
    hint = hfpad[:, 1:65, 1:129]
    def evac_y2(ch, pt):
        rows = slice(1 + 4 * ch, 1 + 4 * ch + 4)
        nc.scalar.activation(hfpad[:, rows, 1:129], pt[...], AF.Gelu,
                             bias=cc['fh_b2v'][...])
    tap_conv(y1pad, cc['fhD2'], [(ii, ii, 1) for ii in range(3)], evac_y2)
    nc.vector.tensor_mul(hint, hint, highpad[:, 1:65, 1:129])
    nc.sync.dma_start(out=hfpad[0:48, 65:66, 1:129], in_=hfpad[64:112, 1:2, 1:129])
    nc.sync.dma_start(out=hfpad[64:112, 0:1, 1:129], in_=hfpad[0:48, 64:65, 1:129])
    tap_pad('hf', hfpad)

    # ============ P4.5: lfa k/v convs (fill the PE while FFT deps settle;
    # reuse the lowwrev/lowrev buffers, whose last readers are done) ====
    kl = a2c.tile([NP, HH, W], BF, tag="s3")
    vl = a2c.tile([NP, HH, W], BF, tag="s4")
    conv_taps(kl, xpad0, cc['lkT'])
    conv_taps(vl, xpad0, cc['lvT'])

    # ============ P7: FFT per 4-channel group ============
    lfD = dram.tile([C, H, W], BF, tag="lfD")

    def transpose_pair(src, out_tag):
        d = scrF.tile([128, 1024], BF, tag="fbig")
        for half in range(2):
            pt = ps_tr.tile([128, 512], BF, tag="tr")
            for c4 in range(4):
                nc.tensor.transpose(
                    pt[:, c4 * 128:(c4 + 1) * 128],
                    src[:, half * 512 + c4 * 128:half * 512 + (c4 + 1) * 128],
                    cc['I128b'][...])
            evac(d[:, half * 512:(half + 1) * 512], pt[...], half)
        return d

    for grp in range(12):
        c0 = grp * 4
        ld = {}
        for nm, dten in (('lo', lowD), ('wr', WreD), ('wi', WimD)):
            t = scrFi.tile([128, 512], BF, tag="fi")
            src = _ap(dten[...], c0 * W, [[C * W, 128], [W, 4], [1, W]])
            nc.sync.dma_start(out=t[...], in_=src)
            ld[nm] = t

        # W path: stage A (contract h), complex input
        sA = scrF.tile([128, 1024], BF, tag="fbig")
        pre = ps_tr.tile([128, 512], F32, tag="tr")
        pim = ps_tr.tile([128, 512], F32, tag="tr")
        nc.tensor.matmul(pre[...], cc['CH'][...], ld['wr'][...],
                         start=True, stop=False)
        nc.tensor.matmul(pim[...], cc['CH'][...], ld['wi'][...],
                         start=True, stop=False)
        nc.tensor.matmul(pre[...], cc['SH'][...], ld['wi'][...],
                         start=False, stop=True)
        nc.tensor.matmul(pim[...], cc['SHn'][...], ld['wr'][...],
                         start=False, stop=True)
        evac(sA[:, 0:512], pre[...], 0)
        evac(sA[:, 512:1024], pim[...], 1)
        sAT = transpose_pair(sA, "sAT")
        # stage B (contract w) -> gelu gates
        gre = ps_tr.tile([128, 512], F32, tag="tr")
        gim = ps_tr.tile([128, 512], F32, tag="tr")
        nc.tensor.matmul(gre[0:65, :], cc['CW'][...], sAT[:, 0:512],
                         start=True, stop=False)
        nc.tensor.matmul(gim[0:65, :], cc['CW'][...], sAT[:, 512:1024],
                         start=True, stop=False)
        nc.tensor.matmul(gre[0:65, :], cc['SW'][...], sAT[:, 512:1024],
                         start=False, stop=True)
        nc.tensor.matmul(gim[0:65, :], cc['SWn'][...], sAT[:, 0:512],
                         start=False, stop=True)
        gate = scr.tile([65, 1024], BF, tag="gz")
        nc.scalar.activation(gate[:, 0:512], gre[0:65, :], AF.Gelu)
        nc.scalar.activation(gate[:, 512:1024], gim[0:65, :], AF.Gelu)

        # low path
        sY = scrF.tile([128, 1024], BF, tag="fbig")
        pyr = ps_tr.tile([128, 512], F32, tag="tr")
        pyi = ps_tr.tile([128, 512], F32, tag="tr")
        nc.tensor.matmul(pyr[...], cc['CH'][...], ld['lo'][...],
                         start=True, stop=True)
        nc.tensor.matmul(pyi[...], cc['SHn'][...], ld['lo'][...],
                         start=True, stop=True)
        evac(sY[:, 0:512], pyr[...], 0)
        evac(sY[:, 512:1024], pyi[...], 1)
        sYT = transpose_pair(sY, "sYT")
        zre = ps_tr.tile([128, 512], F32, tag="tr")
        zim = ps_tr.tile([128, 512], F32, tag="tr")
        nc.tensor.matmul(zre[0:65, :], cc['CW'][...], sYT[:, 0:512],
                         start=True, stop=False)
        nc.tensor.matmul(zim[0:65, :], cc['CW'][...], sYT[:, 512:1024],
                         start=True, stop=False)
        nc.tensor.matmul(zre[0:65, :], cc['SW'][...], sYT[:, 512:1024],
                         start=False, stop=True)
        nc.tensor.matmul(zim[0:65, :], cc['SWn'][...], sYT[:, 0:512],
                         start=False, stop=True)
        zm = scr.tile([65, 1024], BF, tag="gz")
        nc.vector.tensor_mul(zm[:, 0:512], zre[0:65, :], gate[:, 0:512])
        nc.vector.tensor_mul(zm[:, 512:1024], zim[0:65, :],
                             gate[:, 512:1024])

        # inverse: contract fw then fh
        sAB = scrF.tile([128, 1024], BF, tag="fbig")
        am = ps_tr.tile([128, 512], F32, tag="tr")
        bm = ps_tr.tile([128, 512], F32, tag="tr")
        nc.tensor.matmul(am[...], cc['IWcT'][...], zm[:, 0:512],
                         start=True, stop=False)
        nc.tensor.matmul(bm[...], cc['IWsT'][...], zm[:, 0:512],
                         start=True, stop=False)
        nc.tensor.matmul(am[...], cc['IWsT'][...], zm[:, 512:1024],
                         start=False, stop=True)
        nc.tensor.matmul(bm[...], cc['IWcTn'][...], zm[:, 512:1024],
                         start=False, stop=True)
        evac(sAB[:, 0:512], am[...], 0)
        evac(sAB[:, 512:1024], bm[...], 1)
        sABT = transpose_pair(sAB, "sABT")
        xl = ps_tr.tile([128, 512], F32, tag="tr")
        nc.tensor.matmul(xl[...], cc['cH2'][...], sABT[:, 0:512],
                         start=True, stop=False)
        nc.tensor.matmul(xl[...], cc['sH2'][...], sABT[:, 512:1024],
                         start=False, stop=True)
        xst = scr.tile([128, 512], BF, tag="wst")
        nc.vector.tensor_copy(xst[...], xl[...])
        dst = _ap(lfD[...], c0 * HW, [[128, 128], [HW, 4], [1, 128]])
        nc.sync.dma_start(out=dst, in_=xst[...])

    lfpad = pads.tile([NP, 66, 130], BF, tag="p0")
    pad_border(lfpad)
    nc.sync.dma_start(out=lfpad[0:48, 1:66, 1:129], in_=lfD[:, 0:65, :])
    nc.sync.dma_start(out=lfpad[64:112, 0:65, 1:129], in_=lfD[:, 63:128, :])
    if 'lf' in tap_d:
        g.dma_start(out=tap_d['lf'][...], in_=lfD[...])

    # ============ P9/P10: attention ============
    def fga_core(q, k, v, pre, to, tname, wpt, tapn):
        qf = q[...].rearrange("p a b -> p (a b)")
        kf = k[...].rearrange("p a b -> p (a b)")
        # One folded gram G[128, 128]: lhsT = rhs = 128-col strided view of
        # qkt picking (q-half | k-half) with the A2 gaps as ignored junk.
        # G[0:48, 0:48] = qq (halves summed in psum), G[0:48, 64:112] = qk,
        # G[64:112, 64:112] = kk.  LDW is 128-wide -> FWL kicks in.
        G = ps_g.tile([128, 128], F32, tag="gA")
        qkts = {}
        for sc in range(65):
            if sc < 64:
                pt = ps_tr.tile([128, 224], BF, tag="tr")
                nc.tensor.transpose(pt[:, 0:112],
                                    qf[:, 128 * sc:128 * sc + 128],
                                    cc['I128b'][0:112, 0:112])
                nc.tensor.transpose(pt[:, 112:224],
                                    kf[:, 128 * sc:128 * sc + 128],
                                    cc['I128b'][0:112, 0:112])
                qkt_n = scr.tile([128, 256], BF, tag="qkt",
                                 name="qkt%d" % sc)
                # [q-h0 | k-h0 | q-h1 | k-h1] in 64-col blocks so the gram
                # lhsT/rhs are single contiguous 128-col slices (FWL-wide).
                which = sc % 2
                evac(qkt_n[:, 0:48], pt[:, 0:48], which)
                evac(qkt_n[:, 64:112], pt[:, 112:160], which)
                evac(qkt_n[:, 128:176], pt[:, 64:112], 1 - which)
                evac(qkt_n[:, 192:240], pt[:, 176:224], 1 - which)
                qkts[sc] = qkt_n
            if sc == 0:
                continue
            gc = sc - 1
            qkt = qkts.pop(gc)
            for hb in (0, 1):
                ap = qkt[:, 128 * hb:128 * hb + 128]
                nc.tensor.matmul(G[...], ap, ap,
                                 start=(gc == 0 and hb == 0),
                                 stop=(gc == 63 and hb == 1),
                                 skip_group_check=True)

        t48 = sm.tile([48, 48], F32, tag="t48")
        dq = sm.tile([48, 1], F32, tag="dq")
        tk48 = sm.tile([NP, 48], F32, tag="tk48")
        dkv = sm.tile([NP, 1], F32, tag="dkv")
        nc.vector.tensor_mul(t48[...], G[0:48, 0:48], cc['Ia2f'][0:48, :])
        nc.vector.reduce_sum(dq[...], t48[...], AX.X)
        nc.vector.tensor_mul(tk48[64:112, :], G[64:112, 64:112],
                             cc['Ia2f'][64:112, :])
        nc.vector.reduce_sum(dkv[64:112, :], tk48[64:112, :], AX.X)
        nc.scalar.activation(dq[...], dq[...], AF.Sqrt)
        nc.vector.reciprocal(dq[...], dq[...])
        nc.scalar.activation(dkv[64:112, :], dkv[64:112, :], AF.Sqrt)
        nc.vector.reciprocal(dkv[64:112, :], dkv[64:112, :])
        nc.vector.tensor_mul(dq[...], dq[...], cc[tname][...])
        prk = ps_g.tile([1, 48], F32, tag="gsm")
        nc.tensor.matmul(prk[...], dkv[64:112, :], cc['Ia2f'][64:112, :],
                         start=True, stop=True)
        rkrow = sm.tile([1, 48], F32, tag="rkrow")
        nc.vector.tensor_copy(rkrow[...], prk[...])
        rkmat = sm.tile([48, 48], F32, tag="rkmat")
        nc.gpsimd.partition_broadcast(rkmat[...], rkrow[...])

        S = sm.tile([48, 48], F32, tag="S")
        nc.vector.scalar_tensor_tensor(S[...], G[0:48, 64:112], dq[...],
                                       rkmat[...], OP.mult, OP.mult)
        nc.vector.tensor_add(S[...], S[...], cc['mask48'][...])
        nmax = sm.tile([48, 1], F32, tag="nmax")
        nc.vector.reduce_max(nmax[...], S[...], AX.X, negate=True)
        nc.scalar.activation(S[...], S[...], AF.Exp, bias=nmax[...])
        ssum = sm.tile([48, 1], F32, tag="ssum")
        nc.vector.reduce_sum(ssum[...], S[...], AX.X)
        nc.vector.reciprocal(ssum[...], ssum[...])
        nc.vector.tensor_scalar(S[...], S[...], ssum[...], None, OP.mult)
        pm2 = ps_g.tile([48, 48], F32, tag="gsm")
        nc.tensor.matmul(pm2[...], S[...], cc[wpt][...], start=True,
                         stop=True)
        m2 = sm.tile([NP, 64], BF, tag="m2")
        nc.vector.memset(m2[0:48, 48:64], 0.0)
        nc.vector.tensor_copy(m2[0:48, 0:48], pm2[...])
        nc.sync.dma_start(out=m2[64:112, 0:48], in_=m2[0:48, 0:48])

        o = a2c.tile([NP, HH, W], BF, tag=to)
        for ch in range(16):
            pt = ps_mm.tile([NP, 512], F32, tag="mm")
            for p0, mo in ((0, 64), (64, 48)):
                nc.tensor.matmul(pt[p0:p0 + mo, :], m2[p0:p0 + 48, 0:mo],
                                 v[p0:p0 + 48, 4 * ch:4 * ch + 4, :],
                                 start=True, stop=True,
                                 tile_position=(p0, p0),
                                 skip_group_check=True)
            evac(o[:, 4 * ch:4 * ch + 4, :], pt[...], ch % 2)
        tap_a2(tapn, o)
        return o

    qh = a2c.tile([NP, HH, W], BF, tag="s0")
    conv_taps(qh, hfpad, cc['hqT'])
    ho = fga_core(qh, kh, vh, 'h', "s2", 't_h', 'WpT_h', 'ho')
    ql = a2c.tile([NP, HH, W], BF, tag="s0")
    conv_taps(ql, lfpad, cc['lqT'])
    lo = fga_core(ql, kl, vl, 'l', "s5", 't_l', 'WpT_l', 'lo')

    # ============ P11: final projection + bias + residual ============
    for ch in range(16):
        rows = slice(4 * ch, 4 * ch + 4)
        pt = ps_mm.tile([NP, 512], F32, tag="mm")
        for p0, mo in ((0, 64), (64, 48)):
            nc.tensor.matmul(pt[p0:p0 + mo, :],
                             cc['fpw_hT'][p0:p0 + 48, 0:mo],
                             ho[p0:p0 + 48, rows, :], start=True, stop=False,
                             tile_position=(p0, p0), skip_group_check=True)
        for p0, mo in ((0, 64), (64, 48)):
            nc.tensor.matmul(pt[p0:p0 + mo, :],
                             cc['fpw_lT'][p0:p0 + 48, 0:mo],
                             lo[p0:p0 + 48, rows, :], start=False, stop=True,
                             tile_position=(p0, p0), skip_group_check=True)
        ost = scr.tile([NP, 512], F32, tag="ost")
        nc.vector.scalar_tensor_tensor(
            ost[...].rearrange("p (a b) -> p a b", a=4),
            pt[...].rearrange("p (a b) -> p a b", a=4),
            cc['fp_bv'][...],
            xpad0[:, 1 + 4 * ch:5 + 4 * ch, 1:129], OP.add, OP.add)
        nc.sync.dma_start(out=out_d[:, 4 * ch:4 * ch + 4, :],
                          in_=ost[0:48, :])
        nc.sync.dma_start(out=out_d[:, 64 + 4 * ch:64 + 4 * ch + 4, :],
                          in_=ost[64:112, :])


# ======================= host-side runner =======================
_CACHE = {}


def get_module(taps=()):
    key = tuple(sorted(taps))
    if key not in _CACHE:
        _CACHE[key] = build_module(taps)
    return _CACHE[key]


def make_in_maps(inputs, n_cores=8):
    """inputs: full dict per setup_inputs(). Returns per-core in_maps."""
    w = {k: np.asarray(v, np.float32) for k, v in inputs.items()
         if k != 'x'}
    cst = host_consts(w)
    cst_cast = {}
    for k, v in cst.items():
        cst_cast[k] = np.ascontiguousarray(v)
    x = np.asarray(inputs['x'], np.float32)
    maps = []
    for core in range(n_cores):
        m = dict(cst_cast)
        m['x'] = np.ascontiguousarray(x[core])
        maps.append(m)
    return maps


def kernel(**inputs):
    """Full-input entry: shards batch over 8 NeuronCores, returns full output.

    inputs: exactly the dict produced by setup_inputs() (numpy arrays).
    """
    from concourse.bass_utils import run_bass_kernel_spmd
    x = np.asarray(inputs['x'], np.float32)
    n_cores = x.shape[0]
    nc = get_module(())
    in_maps = make_in_maps(inputs, n_cores=n_cores)
    res = run_bass_kernel_spmd(nc, in_maps, core_ids=list(range(n_cores)),
                               trace=False)
    out = np.stack([res.results[c]["out"] for c in range(n_cores)])
    return out.astype(np.float32)

